# revision 29
# baseline (speedup 1.0000x reference)
"""Causal self-attention with RoPE on 8 Trainium2 NeuronCores.

Problem: B=4, T=2048, C=1024, NH=16, D=64. y = proj(attn(rope(qkv(x)))).

Sharding: core = (batch b, head-group hg): 4 batches x 2 groups of 8 heads.
Each core computes its 8 heads' attention for its batch plus the partial
output projection over its 512 head-channels; the host sums the two
partials per batch and adds b_proj.

On-device layout is "transposed" throughout ([feature partitions, token
free-dim]) so no on-chip transposes are needed:
  - qT/kT produced as [d, t] directly from the QKV matmul
  - RoPE rotate_half done with a constant rotation matmul + elementwise
  - scoresT[kv, q] = kT.T-slice @ qT-slice per 128-kv tile
  - softmax denominator via a ones-column appended to V (free on PE)
  - PV gives yT[d, q]; normalization via reciprocal + partition broadcast
  - output projection consumes yT tiles directly as the stationary operand
All matmuls run in float32r (full-rate; ~2e-4 rel err measured on HW).
"""
import math
from contextlib import ExitStack

import numpy as np

import concourse.bass as bass
import concourse.tile as tile
from concourse import bacc, mybir
from concourse.bass_utils import run_bass_kernel_spmd

B, T, C, NH, D = 4, 2048, 1024, 16, 64
P = 128                 # partitions
GN = 512                # token-group size
TG = T // GN            # 4 token groups
KT = C // P             # 8 contraction tiles over C
NCORES = 8
HPC = 8                 # heads per core
f32 = mybir.dt.float32
f32r = mybir.dt.float32r
AF = mybir.ActivationFunctionType

_NC_CACHE = None


def _body(ctx, tc, xT, wqkT, wvT, wpT, bqk, bv, cosT, sinT, rmat, dmask,
          onescol, outp):
    nc = tc.nc

    const = ctx.enter_context(tc.tile_pool(name="const", bufs=1))
    resid = ctx.enter_context(tc.tile_pool(name="resid", bufs=1))
    xpool = ctx.enter_context(tc.tile_pool(name="xpool", bufs=8))
    cspool = ctx.enter_context(tc.tile_pool(name="cspool", bufs=1))
    rawp = ctx.enter_context(tc.tile_pool(name="rawp", bufs=2))
    tmpp = ctx.enter_context(tc.tile_pool(name="tmpp", bufs=2))
    attp = ctx.enter_context(tc.tile_pool(name="attp", bufs=2))
    bcp = ctx.enter_context(tc.tile_pool(name="bcp", bufs=2))
    rcp = ctx.enter_context(tc.tile_pool(name="rcp", bufs=2))
    rsp = ctx.enter_context(tc.tile_pool(name="rsp", bufs=2))
    outsb = ctx.enter_context(tc.tile_pool(name="outsb", bufs=1))
    psmm = ctx.enter_context(tc.tile_pool(name="psmm", bufs=2, space="PSUM"))
    pssc = ctx.enter_context(tc.tile_pool(name="pssc", bufs=2, space="PSUM"))
    psy = ctx.enter_context(tc.tile_pool(name="psy", bufs=2, space="PSUM"))

    # ---- constants / resident tensors ----
    wqk_t = []
    for k in range(KT):
        w_ = const.tile([P, 1024], f32r, tag=f"wqk{k}", name=f"wqk{k}")
        nc.sync.dma_start(w_[:], wqkT[k * P:(k + 1) * P, :])
        wqk_t.append(w_)
    # wv/wp loads are deferred into the g=0 body so the startup DMA
    # bandwidth goes to x/wqk first (lower scheduling priority).
    wv_t = [const.tile([P, 512], f32r, tag=f"wv{k}", name=f"wv{k}")
            for k in range(KT)]
    wp_t = [const.tile([P, 1024], f32r, tag=f"wp{k}", name=f"wp{k}")
            for k in range(4)]
    rmat_t = const.tile([P, P], f32r, tag="rmat", name="rmat_t")
    nc.sync.dma_start(rmat_t[:], rmat[:])
    dmask_t = const.tile([P, P], f32, tag="dmask", name="dmask_t")
    nc.sync.dma_start(dmask_t[:], dmask[:])
    bqk_t = const.tile([P, 8], f32, tag="bqk", name="bqk_t")
    nc.sync.dma_start(bqk_t[:], bqk[:])
    bv_t = const.tile([1, 512], f32r, tag="bv", name="bv_t")
    nc.sync.dma_start(bv_t[:], bv[:])
    ones_t = const.tile([P, P], f32r, tag="ones", name="ones_t")
    nc.sync.dma_start(ones_t[:], onescol[:])

    kT_t = []
    for p in range(4):
        k_ = resid.tile([P, T], f32r, tag=f"kT{p}", name=f"kT{p}")
        kT_t.append(k_)
    # vplus layout: [128 tok, tt(16) x head(8) x (64 d + 1 ones)]
    vplus = resid.tile([P, 16 * HPC * 65], f32r, tag="vplus", name="vplus")
    vp4 = vplus[:].rearrange("p (t h e) -> p t h e", t=16, h=HPC)
    # ones columns via strided memset (1.0f bit pattern through an f32 view)
    nc.gpsimd.memset(vp4[:, :, :, 64:65].bitcast(f32), 1.0)
    qT_g = []
    for p in range(4):
        q_ = resid.tile([P, GN], f32r, tag=f"qT{p}", name=f"qT{p}")
        qT_g.append(q_)
    # double-buffered so proj(g-1) reads are independent of B(g) writes
    yT_sets = []
    for e in range(2):
        yT_sets.append([resid.tile([P, GN], f32r, tag=f"yT{e}_{p}",
                                   name=f"yT{e}_{p}") for p in range(4)])

    def _proj(g_):
        yT_g = yT_sets[g_ % 2]
        # output projection for group g_ (partial over 512 channels).
        # Emitted AFTER the next group's QKV phase: its first MM waits on
        # the slowest pair's normalization, and the in-order PE queue
        # must have independent work ahead of that stall.
        if g_ == 0:
            for k in range(4):
                nc.sync.dma_start(wp_t[k][:], wpT[k * P:(k + 1) * P, :])
        for tt in range(4):
            for n in range(2):
                o_ps = psy.tile([P, GN], f32, tag="y",
                                name=f"ops{g_}_{tt}_{n}")
                for i, p in enumerate((3, 2, 1, 0)):
                    nc.tensor.matmul(o_ps[:], yT_g[p][:, tt * P:(tt + 1) * P],
                                     wp_t[p][:, n * GN:(n + 1) * GN],
                                     start=(i == 0), stop=(i == 3))
                o_sb = outsb.tile([P, GN], f32, tag="osb",
                                  name=f"osb{g_}_{tt}_{n}")
                nc.vector.tensor_copy(o_sb[:], o_ps[:])
                nc.sync.dma_start(
                    outp[g_ * GN + tt * P: g_ * GN + (tt + 1) * P,
                         n * GN:(n + 1) * GN], o_sb[:])

    for g in range(TG):
        gsl = slice(g * GN, (g + 1) * GN)
        # ---- per-group loads ----
        cos_t = cspool.tile([P, GN], f32, tag="cos", name=f"cos{g}")
        nc.sync.dma_start(cos_t[:], cosT[:, gsl])
        sin_t = cspool.tile([P, GN], f32, tag="sin", name=f"sin{g}")
        nc.sync.dma_start(sin_t[:], sinT[:, gsl])
        x_t = []
        for k in range(KT):
            x_ = xpool.tile([P, GN], f32r, tag="xt", name=f"xt{g}_{k}")
            nc.sync.dma_start(x_[:], xT[k * P:(k + 1) * P, gsl])
            x_t.append(x_)

        # ---- QKV projection for q/k feats (8 tiles of 128 feats) + RoPE ----
        for f in range(8):
            mm_ps = psmm.tile([P, GN], f32, tag="mm", name=f"qkps{g}_{f}")
            for k in range(KT):
                nc.tensor.matmul(mm_ps[:], wqk_t[k][:, f * P:(f + 1) * P],
                                 x_t[k][:], start=(k == 0), stop=(k == KT - 1))
            raw = rawp.tile([P, GN], f32r, tag="raw", name=f"raw{g}_{f}")
            nc.vector.tensor_scalar_add(raw[:], mm_ps[:], bqk_t[:, f:f + 1])
            rot_ps = psmm.tile([P, GN], f32, tag="mm", name=f"rotps{g}_{f}")
            nc.tensor.matmul(rot_ps[:], rmat_t[:], raw[:], start=True, stop=True)
            tmp = tmpp.tile([P, GN], f32, tag="tmp", name=f"tmp{g}_{f}")
            nc.vector.tensor_mul(tmp[:], rot_ps[:], sin_t[:])
            dst = qT_g[f][:] if f < 4 else kT_t[f - 4][:, gsl]
            nc.vector.tensor_mul(dst, raw[:], cos_t[:])
            nc.vector.tensor_add(dst, dst, tmp[:])

        # ---- V projection (+ b_v via K=1 matmul) into vplus ----
        if g == 0:
            for k in range(KT):
                nc.sync.dma_start(wv_t[k][:], wvT[k * P:(k + 1) * P, :])
        for tt in range(4):
            ttg = g * 4 + tt
            v_ps = psmm.tile([P, GN], f32, tag="mm", name=f"vps{g}_{tt}")
            for k in range(KT):
                nc.tensor.matmul(v_ps[:], x_t[k][:, tt * P:(tt + 1) * P],
                                 wv_t[k][:], start=(k == 0), stop=False)
            nc.tensor.matmul(v_ps[:], ones_t[0:1, :], bv_t[:],
                             start=False, stop=True)
            nc.vector.tensor_copy(vp4[:, ttg, :, 0:64],
                                  v_ps[:].rearrange("p (h e) -> p h e", h=HPC))

        if g > 0:
            _proj(g - 1)

        # ---- attention: head pairs, even/odd fused in one 2-bank PSUM ----
        njt = 4 * g + 4                      # kv tiles for this q-group

        yT_g = yT_sets[g % 2]

        def _finish_norm(p_, rcrows):
            # broadcasts + final normalize muls for pair p_; emitted one
            # pair late so the reciprocal DMA roundtrip latency hides
            # behind the next pair's attention instead of stalling the
            # in-order gpsimd queue ahead of its causal masks.
            bcb = bcp.tile([P, GN], f32, tag="bcb", name=f"bcb{g}_{p_}")
            nc.gpsimd.partition_broadcast(bcb[0:64, :], rcrows[0][:])
            nc.vector.tensor_mul(yT_g[p_][0:64, :], yT_g[p_][0:64, :],
                                 bcb[0:64, :])
            bcb2 = bcp.tile([P, GN], f32, tag="bcb", name=f"bcb2{g}_{p_}")
            nc.gpsimd.partition_broadcast(bcb2[0:64, :], rcrows[1][:])
            nc.scalar.dma_start(bcb2[64:128, :], bcb2[0:64, :])
            nc.vector.tensor_mul(yT_g[p_][64:128, :],
                                 yT_g[p_][64:128, :], bcb2[64:128, :])

        pending_norm = None
        for p in range(4):
            yps = [None, None]
            for s in range(2):
                yps[s] = psy.tile([65, GN], f32, tag="y", name=f"yps{g}_{p}_{s}")
            # software-pipelined: QK/exp for tile j+1 are issued BEFORE the
            # PV of tile j so the in-order PE queue never stalls on exp.
            prev_a2 = None

            # diagonal tiles first: their mask latency hides behind the
            # following full tiles, and the pair ends on a short chain.
            jorder = list(range(4 * g, njt)) + list(range(0, 4 * g))

            def _pv(ji_, a2_):
                for s in range(2):
                    h = 2 * p + s
                    nc.tensor.matmul(yps[s][:], vp4[:, jorder[ji_], h, :],
                                     a2_[:, s * GN:(s + 1) * GN],
                                     start=(ji_ == 0), stop=(ji_ == njt - 1))

            for ji in range(njt):
                j = jorder[ji]
                r = j - 4 * g                # >=0 on diagonal tiles
                c0 = max(r, 0) * P           # first valid q column
                sc2 = pssc.tile([P, 2 * GN], f32, tag="sc",
                                name=f"sc{g}_{p}_{j}")
                for s in range(2):
                    hb = s * 64
                    nc.tensor.matmul(
                        sc2[:, s * GN + c0:(s + 1) * GN],
                        kT_t[p][hb:hb + 64, j * P:(j + 1) * P],
                        qT_g[p][hb:hb + 64, c0:GN],
                        start=True, stop=True)
                a2 = attp.tile([P, 2 * GN], f32r, tag="att",
                               name=f"att{g}_{p}_{j}")
                sc2v = sc2[:].rearrange("p (s q) -> p s q", s=2)
                a2v = a2[:].rearrange("p (s q) -> p s q", s=2)
                if c0 > 0:
                    nc.gpsimd.memset(a2v[:, :, 0:c0].bitcast(f32), 0.0)
                nc.scalar.activation(a2v[:, :, c0:GN], sc2v[:, :, c0:GN],
                                     AF.Exp, scale=1.0 / math.sqrt(D))
                if r >= 0:
                    nc.gpsimd.tensor_mul(a2[:, c0:c0 + P],
                                         a2[:, c0:c0 + P], dmask_t[:])
                    nc.gpsimd.tensor_mul(a2[:, GN + c0:GN + c0 + P],
                                         a2[:, GN + c0:GN + c0 + P], dmask_t[:])
                if prev_a2 is not None:
                    _pv(ji - 1, prev_a2)
                prev_a2 = a2
            _pv(njt - 1, prev_a2)
            if pending_norm is not None:
                _finish_norm(*pending_norm)
            # tail: copy y-body + rowsum row out of PSUM fast (frees the
            # yps banks so the next pair's PV can start), then one cheap
            # batched reciprocal on a DMA-transposed [128, 8] layout.
            rs_p = rsp.tile([P, 8], f32, tag="rs", name=f"rs{g}_{p}")
            for s in range(2):
                hb = s * 64
                nc.vector.tensor_copy(yT_g[p][hb:hb + 64, :], yps[s][0:64, :])
                rrow = rcp.tile([1, GN], f32, tag="rrow", name=f"rrow{g}_{p}_{s}")
                nc.vector.tensor_copy(rrow[:], yps[s][64:65, :])
                nc.scalar.dma_start(rs_p[:, s * 4:(s + 1) * 4], rrow[:])
            rc_p = rsp.tile([P, 8], f32, tag="rc", name=f"rcp{g}_{p}")
            nc.vector.reciprocal(rc_p[:], rs_p[:])
            rcrows = []
            for s in range(2):
                rcrow = rcp.tile([1, GN], f32, tag="rcrow", bufs=2,
                                 name=f"rcrow{g}_{p}_{s}")
                nc.scalar.dma_start(rcrow[:], rc_p[:, s * 4:(s + 1) * 4])
                rcrows.append(rcrow)
            pending_norm = (p, rcrows)
        _finish_norm(*pending_norm)

    _proj(TG - 1)


def build_nc():
    nc = bacc.Bacc("TRN2", target_bir_lowering=False, debug=False,
                   num_devices=NCORES)
    xT = nc.dram_tensor("xT", [C, T], f32r, kind="ExternalInput").ap()
    wqkT = nc.dram_tensor("wqkT", [C, 1024], f32r, kind="ExternalInput").ap()
    wvT = nc.dram_tensor("wvT", [C, 512], f32r, kind="ExternalInput").ap()
    wpT = nc.dram_tensor("wpT", [512, 1024], f32r, kind="ExternalInput").ap()
    bqk = nc.dram_tensor("bqk", [P, 8], f32, kind="ExternalInput").ap()
    bv = nc.dram_tensor("bv", [1, 512], f32r, kind="ExternalInput").ap()
    cosT = nc.dram_tensor("cosT", [P, T], f32, kind="ExternalInput").ap()
    sinT = nc.dram_tensor("sinT", [P, T], f32, kind="ExternalInput").ap()
    rmat = nc.dram_tensor("rmat", [P, P], f32r, kind="ExternalInput").ap()
    dmask = nc.dram_tensor("dmask", [P, P], f32, kind="ExternalInput").ap()
    onescol = nc.dram_tensor("onescol", [P, P], f32r, kind="ExternalInput").ap()
    outp = nc.dram_tensor("outp", [T, C], f32, kind="ExternalOutput").ap()
    with tile.TileContext(nc) as tc, \
            nc.allow_low_precision(reason="f32r matmul operands"):
        with ExitStack() as ctx:
            _body(ctx, tc, xT, wqkT, wvT, wpT, bqk, bv, cosT, sinT, rmat,
                  dmask, onescol, outp)
    nc.compile()
    return nc


def _host_inputs(x, w_attn, b_attn, w_proj, cos, sin):
    """Build the 8 per-core input dicts."""
    # rotation matrix: ROT @ q == rotate_half(q) in [d] space
    rot = np.zeros((D, D), np.float32)
    for d_ in range(32):
        rot[d_, d_ + 32] = -1.0
        rot[d_ + 32, d_] = 1.0
    rmat = np.zeros((P, P), np.float32)
    rmat[0:D, 0:D] = rot.T
    rmat[D:P, D:P] = rot.T
    dmask = np.triu(np.ones((P, P), np.float32))
    onescol = np.ones((P, P), np.float32)
    cosT2 = np.ascontiguousarray(
        np.concatenate([cos[0].T, cos[0].T], axis=0))      # [128, T]
    sinT2 = np.ascontiguousarray(np.concatenate([sin[0].T, sin[0].T], axis=0))

    in_maps = []
    for core in range(NCORES):
        b = core // 2
        hg = core % 2
        h0 = hg * HPC
        qrows = slice(h0 * D, (h0 + HPC) * D)              # 512 rows
        krows = slice(C + h0 * D, C + (h0 + HPC) * D)
        vrows = slice(2 * C + h0 * D, 2 * C + (h0 + HPC) * D)
        wqk = np.concatenate([w_attn[qrows], w_attn[krows]], axis=0)  # [1024, C]
        bqk_np = np.concatenate([b_attn[qrows], b_attn[krows]])       # [1024]
        in_maps.append({
            "xT": np.ascontiguousarray(x[b].T),                        # [C, T]
            "wqkT": np.ascontiguousarray(wqk.T),                       # [C, 1024]
            "wvT": np.ascontiguousarray(w_attn[vrows].T),              # [C, 512]
            "wpT": np.ascontiguousarray(w_proj[:, h0 * D:(h0 + HPC) * D].T),
            "bqk": np.ascontiguousarray(bqk_np.reshape(8, P).T),       # [128, 8]
            "bv": np.ascontiguousarray(b_attn[vrows].reshape(1, 512)),
            "cosT": cosT2,
            "sinT": sinT2,
            "rmat": rmat,
            "dmask": dmask,
            "onescol": onescol,
        })
    return in_maps


def kernel(x, w_attn, b_attn, w_proj, b_proj, cos, sin):
    global _NC_CACHE
    x = np.asarray(x, np.float32)
    w_attn = np.asarray(w_attn, np.float32)
    b_attn = np.asarray(b_attn, np.float32)
    w_proj = np.asarray(w_proj, np.float32)
    b_proj = np.asarray(b_proj, np.float32)
    cos = np.asarray(cos, np.float32)
    sin = np.asarray(sin, np.float32)

    if _NC_CACHE is None:
        _NC_CACHE = build_nc()
    nc = _NC_CACHE
    in_maps = _host_inputs(x, w_attn, b_attn, w_proj, cos, sin)
    res = run_bass_kernel_spmd(nc, in_maps, core_ids=list(range(NCORES)))
    parts = [res.results[i]["outp"] for i in range(NCORES)]
    out = np.empty((B, T, C), np.float32)
    for b in range(B):
        out[b] = parts[2 * b] + parts[2 * b + 1] + b_proj
    return out
